# revision 1
# baseline (speedup 1.0000x reference)
"""DCNv2 (deformable conv v2) Trainium2 kernel.

Problem: x[4,256,64,64], offset[4,18,64,64], mask[4,9,64,64],
weight[256,256,3,3], bias[256] -> out[4,256,64,64].  3x3, stride 1, pad 1.

Sharding: 8 cores = 4 images x 2 pixel-halves (32 output rows each).

Per-core pipeline (all compute on device):
  1. DVE computes bilinear corner weights + patch indices from offsets
     (validity and mask folded into the 4 corner weights).
  2. SWDGE indirect DMA gathers 2x2xC patches (bf16, channel-last,
     patch-materialized in DRAM) -> G[p=128, k=9, r=2, s=2, c=256].
  3. DVE multiplies by corner weights and reduces over the 4 corners
     -> valT[p, (k,c)] (im2col tile, pixel-major).
  4. PE transposes valT -> val[(k,c), p] chunks in PSUM, ACT copies to SBUF.
  5. PE matmul: out[o, p] += wT[(k,c), o]^T @ val[(k,c), p], 18 chunks of 128,
     N=512 (4 pixel-tiles per group), fp32 PSUM accumulate; ACT adds bias.
"""

import sys

sys.path.insert(0, "/opt/trn_rl_repo")

import numpy as np
import ml_dtypes
from contextlib import ExitStack

import concourse.bass as bass
import concourse.tile as tile
from concourse import bacc, mybir
from concourse.masks import make_identity

BF16 = mybir.dt.bfloat16
F32 = mybir.dt.float32
I32 = mybir.dt.int32

B, C, H, W = 4, 256, 64, 64
O, K = 256, 9
ROWS = 32               # output rows per core
P = ROWS * W            # pixels per core = 2048
NT = P // 128           # 16 tiles of 128 pixels
NCHUNK = 2 * K          # 18 contraction chunks of 128
GROUP = 4               # pixel-tiles per matmul group (N=512)

_BIG = 12582912.0       # 1.5*2^23: fp32 RNE rounding trick (valid for |x| < 2^22)


def build_nc(dbg=False):
    nc = bacc.Bacc("TRN2", target_bir_lowering=False, debug=False)

    xp = nc.dram_tensor("xp", [H * W, 1024], BF16, kind="ExternalInput")
    offy = nc.dram_tensor("offy", [128, NT, K], F32, kind="ExternalInput")
    offx = nc.dram_tensor("offx", [128, NT, K], F32, kind="ExternalInput")
    msk = nc.dram_tensor("msk", [128, NT, K], F32, kind="ExternalInput")
    basey = nc.dram_tensor("basey", [128, NT, K], F32, kind="ExternalInput")
    basex = nc.dram_tensor("basex", [128, NT, K], F32, kind="ExternalInput")
    wt = nc.dram_tensor("wt", [NCHUNK, 128, O], BF16, kind="ExternalInput")
    biasd = nc.dram_tensor("biasd", [128, 2], F32, kind="ExternalInput")
    outd = nc.dram_tensor("outd", [2, 128, P], F32, kind="ExternalOutput")
    if dbg:
        g_d = nc.dram_tensor("g_d", [128, K * 1024], BF16, kind="ExternalOutput")
        valt_d = nc.dram_tensor("valt_d", [128, NCHUNK * 128], BF16, kind="ExternalOutput")
        wint_d = nc.dram_tensor("wint_d", [128, NT * K * 4], BF16, kind="ExternalOutput")
        qi_d = nc.dram_tensor("qi_d", [128, NT * K], I32, kind="ExternalOutput")

    with tile.TileContext(nc) as tc, ExitStack() as ctx:
        s = ctx.enter_context(tc.tile_pool(name="singles", bufs=1))
        gpool = ctx.enter_context(tc.tile_pool(name="g", bufs=3))
        hpool = ctx.enter_context(tc.tile_pool(name="h", bufs=2))
        vpool = ctx.enter_context(tc.tile_pool(name="v", bufs=2))
        ppool = ctx.enter_context(tc.tile_pool(name="pt", bufs=1, space="PSUM"))
        mpool = ctx.enter_context(tc.tile_pool(name="mm", bufs=4, space="PSUM"))
        v4pool = ctx.enter_context(tc.tile_pool(name="v4", bufs=2))
        opool = ctx.enter_context(tc.tile_pool(name="o", bufs=4))

        FR = NT * K  # 144 free elems for the plane computations

        pl_n = [0]

        def newpl():
            pl_n[0] += 1
            return s.tile([128, NT, K], F32, name=f"plane{pl_n[0]}")

        # ---- load small inputs ----
        offy_s = s.tile([128, NT, K], F32)
        offx_s = s.tile([128, NT, K], F32)
        msk_s = s.tile([128, NT, K], F32)
        basey_s = s.tile([128, NT, K], F32)
        basex_s = s.tile([128, NT, K], F32)
        nc.sync.dma_start(out=offy_s[:], in_=offy[:, :, :])
        nc.sync.dma_start(out=offx_s[:], in_=offx[:, :, :])
        nc.sync.dma_start(out=msk_s[:], in_=msk[:, :, :])
        nc.sync.dma_start(out=basey_s[:], in_=basey[:, :, :])
        nc.sync.dma_start(out=basex_s[:], in_=basex[:, :, :])

        wt_sb = s.tile([128, NCHUNK, O], BF16)
        nc.sync.dma_start(out=wt_sb[:], in_=wt[:, :, :].rearrange("j c m -> c j m"))
        bias_sb = s.tile([128, 2], F32)
        nc.sync.dma_start(out=bias_sb[:], in_=biasd[:, :])

        ident = s.tile([128, 128], BF16)
        make_identity(nc, ident[:])

        # ---- bilinear weight / index planes ----
        def axis_planes(off_s, base_s):
            """returns (frac-low lo, frac-high hi=1-lo, a=1[f==-1], b=1[0<=f<=62],
            c=1[f==63], sf=clip(f,0,62)) for f=floor(off+base)."""
            pos = newpl()
            nc.vector.tensor_add(out=pos[:], in0=off_s[:], in1=base_s[:])
            r0 = newpl()
            nc.vector.tensor_scalar_add(out=r0[:], in0=pos[:], scalar1=_BIG)
            rnd = newpl()
            nc.vector.tensor_scalar(
                out=rnd[:], in0=r0[:], scalar1=-_BIG, scalar2=None, op0=mybir.AluOpType.add
            )
            g = newpl()
            nc.vector.tensor_tensor(
                out=g[:], in0=rnd[:], in1=pos[:], op=mybir.AluOpType.is_gt
            )
            f = newpl()
            nc.vector.tensor_tensor(
                out=f[:], in0=rnd[:], in1=g[:], op=mybir.AluOpType.subtract
            )
            lo = newpl()
            nc.vector.tensor_tensor(
                out=lo[:], in0=pos[:], in1=f[:], op=mybir.AluOpType.subtract
            )
            hi = newpl()
            nc.vector.tensor_scalar(
                out=hi[:], in0=lo[:], scalar1=-1.0, scalar2=1.0,
                op0=mybir.AluOpType.mult, op1=mybir.AluOpType.add,
            )
            a = newpl()
            nc.vector.tensor_scalar(
                out=a[:], in0=f[:], scalar1=-1.0, scalar2=None, op0=mybir.AluOpType.is_equal
            )
            b0 = newpl()
            nc.vector.tensor_scalar(
                out=b0[:], in0=f[:], scalar1=-1.0, scalar2=None, op0=mybir.AluOpType.is_gt
            )
            b1 = newpl()
            nc.vector.tensor_scalar(
                out=b1[:], in0=f[:], scalar1=63.0, scalar2=None, op0=mybir.AluOpType.is_lt
            )
            b = newpl()
            nc.vector.tensor_tensor(
                out=b[:], in0=b0[:], in1=b1[:], op=mybir.AluOpType.mult
            )
            cc = newpl()
            nc.vector.tensor_scalar(
                out=cc[:], in0=f[:], scalar1=63.0, scalar2=None, op0=mybir.AluOpType.is_equal
            )
            sf = newpl()
            nc.vector.tensor_scalar(
                out=sf[:], in0=f[:], scalar1=0.0, scalar2=62.0,
                op0=mybir.AluOpType.max, op1=mybir.AluOpType.min,
            )
            return lo, hi, a, b, cc, sf

        ly, hy, ay, by, cy, sy = axis_planes(offy_s, basey_s)
        lx, hx, ax, bx, cx, sx = axis_planes(offx_s, basex_s)

        def blend(w_hi, ind_b, w_lo, ind_a):
            """w_hi*ind_b + w_lo*ind_a"""
            m0 = newpl()
            nc.vector.tensor_tensor(
                out=m0[:], in0=w_hi[:], in1=ind_b[:], op=mybir.AluOpType.mult
            )
            m1 = newpl()
            nc.vector.tensor_tensor(
                out=m1[:], in0=w_lo[:], in1=ind_a[:], op=mybir.AluOpType.mult
            )
            r = newpl()
            nc.vector.tensor_add(out=r[:], in0=m0[:], in1=m1[:])
            return r

        v0 = blend(hy, by, ly, ay)   # weight of patch row 0
        v1 = blend(ly, by, hy, cy)   # weight of patch row 1
        u0 = blend(hx, bx, lx, ax)
        u1 = blend(lx, bx, hx, cx)

        vm0 = newpl()
        nc.vector.tensor_tensor(
            out=vm0[:], in0=v0[:], in1=msk_s[:], op=mybir.AluOpType.mult
        )
        vm1 = newpl()
        nc.vector.tensor_tensor(
            out=vm1[:], in0=v1[:], in1=msk_s[:], op=mybir.AluOpType.mult
        )

        wint = s.tile([128, NT, K, 4], BF16)
        for i, (vv, uu) in enumerate(((vm0, u0), (vm0, u1), (vm1, u0), (vm1, u1))):
            nc.vector.tensor_tensor(
                out=wint[:, :, :, i], in0=vv[:], in1=uu[:], op=mybir.AluOpType.mult
            )

        qf = newpl()
        nc.vector.scalar_tensor_tensor(
            out=qf[:], in0=sy[:], scalar=64.0, in1=sx[:],
            op0=mybir.AluOpType.mult, op1=mybir.AluOpType.add,
        )
        qi = s.tile([128, NT, K], I32)
        nc.vector.tensor_copy(out=qi[:], in_=qf[:])

        # ---- main loop over 16 pixel-tiles ----
        val4 = None
        for t in range(NT):
            if t % GROUP == 0:
                val4 = v4pool.tile([128, NCHUNK, GROUP * 128], BF16)

            g = gpool.tile([128, K, 2, 2, C], BF16)
            for k in range(K):
                nc.gpsimd.indirect_dma_start(
                    out=g[:, k].rearrange("p r s c -> p (r s c)"),
                    out_offset=None,
                    in_=xp[:, :],
                    in_offset=bass.IndirectOffsetOnAxis(ap=qi[:, t, k : k + 1], axis=0),
                )

            h = hpool.tile([128, K, C, 4], BF16)
            for r in range(2):
                w_b = wint[:, t, :, 2 * r : 2 * r + 2].to_broadcast([128, K, 2, C])
                nc.vector.tensor_tensor(
                    out=h[:, :, :, 2 * r : 2 * r + 2].rearrange("p k c s -> p k s c"),
                    in0=g[:, :, r, :, :],
                    in1=w_b,
                    op=mybir.AluOpType.mult,
                )

            if dbg and t == 0:
                nc.sync.dma_start(out=g_d[:, :], in_=g[:].rearrange("p k r s c -> p (k r s c)"))
            valt = vpool.tile([128, NCHUNK * 128], BF16)
            with nc.allow_low_precision("bf16 4-term corner sum"):
                nc.vector.tensor_reduce(
                    out=valt[:],
                    in_=h[:, :, :, :],
                    axis=mybir.AxisListType.X,
                    op=mybir.AluOpType.add,
                )

            if dbg and t == 0:
                nc.sync.dma_start(out=valt_d[:, :], in_=valt[:])
                nc.sync.dma_start(out=wint_d[:, :], in_=wint[:].rearrange("p a b c -> p (a b c)"))
                nc.sync.dma_start(out=qi_d[:, :], in_=qi[:].rearrange("p a b -> p (a b)"))
            pst = ppool.tile([128, NCHUNK, 128], BF16, space="PSUM")
            for j in range(NCHUNK):
                nc.tensor.transpose(
                    out=pst[:, j, :],
                    in_=valt[:, j * 128 : (j + 1) * 128],
                    identity=ident[:],
                )
            nc.scalar.copy(
                out=val4[:, :, (t % GROUP) * 128 : (t % GROUP + 1) * 128],
                in_=pst[:, :, :],
            )

            if t % GROUP == GROUP - 1:
                gt = t // GROUP
                for oh in range(2):
                    acc = mpool.tile([128, GROUP * 128], F32, space="PSUM")
                    for j in range(NCHUNK):
                        nc.tensor.matmul(
                            out=acc[:],
                            lhsT=wt_sb[:, j, oh * 128 : (oh + 1) * 128],
                            rhs=val4[:, j, :],
                            start=(j == 0),
                            stop=(j == NCHUNK - 1),
                        )
                    outsb = opool.tile([128, GROUP * 128], F32)
                    nc.scalar.activation(
                        out=outsb[:],
                        in_=acc[:],
                        func=mybir.ActivationFunctionType.Identity,
                        bias=bias_sb[:, oh : oh + 1],
                        scale=1.0,
                    )
                    nc.sync.dma_start(
                        out=outd[oh, :, gt * GROUP * 128 : (gt + 1) * GROUP * 128],
                        in_=outsb[:],
                    )

    nc.compile()
    return nc


_NC = None


def get_nc():
    global _NC
    if _NC is None:
        _NC = build_nc()
    return _NC


def make_in_maps(x, offset, mask, weight, bias):
    """host-side shard prep: pure layout work (transpose/pad/cast)."""
    x = np.asarray(x, np.float32)
    offset = np.asarray(offset, np.float32)
    mask = np.asarray(mask, np.float32)
    weight = np.asarray(weight, np.float32)
    bias = np.asarray(bias, np.float32)

    # patch-materialized channel-last images: xp[qy*64+qx] = x_pad[qy:qy+2, qx:qx+2, :]
    xps = []
    for b in range(B):
        xc = np.zeros((H + 1, W + 1, C), np.float32)
        xc[:H, :W] = x[b].transpose(1, 2, 0)
        pt = np.lib.stride_tricks.sliding_window_view(xc, (2, 2), axis=(0, 1))
        # pt: [H, W, C, 2, 2] -> [H, W, 2(r), 2(s), C]
        pt = pt.transpose(0, 1, 3, 4, 2).reshape(H * W, 1024)
        xps.append(pt.astype(ml_dtypes.bfloat16))

    # weight chunks: wt[j=(k*2+ch)] = weight[:, ch*128:(ch+1)*128, k].T  [c128, O]
    wr = weight.reshape(O, C, K)
    wt = np.empty((NCHUNK, 128, O), np.float32)
    for k in range(K):
        for ch in range(2):
            wt[k * 2 + ch] = wr[:, ch * 128 : (ch + 1) * 128, k].T
    wt = wt.astype(ml_dtypes.bfloat16)

    biasd = bias.reshape(2, 128).T.copy()  # [128, 2] col=ohalf

    ky = np.repeat(np.arange(3), 3).astype(np.float32)  # tap_y[k]
    kx = np.tile(np.arange(3), 3).astype(np.float32)

    def to_dev(v):  # [K, P] -> [128, NT, K]
        return np.ascontiguousarray(
            v.reshape(K, NT, 128).transpose(2, 1, 0)
        ).astype(np.float32)

    in_maps = []
    for core in range(8):
        b, ph = core // 2, core % 2
        rows = slice(ph * ROWS, (ph + 1) * ROWS)
        offb = offset[b].reshape(K, 2, H, W)[:, :, rows, :].reshape(K, 2, P)
        mb_ = mask[b].reshape(K, H, W)[:, rows, :].reshape(K, P)
        ii = np.arange(P, dtype=np.float32) // W + ph * ROWS
        jj = np.arange(P, dtype=np.float32) % W
        bsy = ii[None, :] + ky[:, None] - 1.0  # [K, P]
        bsx = jj[None, :] + kx[:, None] - 1.0
        in_maps.append(
            {
                "xp": xps[b],
                "offy": to_dev(offb[:, 0]),
                "offx": to_dev(offb[:, 1]),
                "msk": to_dev(mb_),
                "basey": to_dev(bsy),
                "basex": to_dev(bsx),
                "wt": wt,
                "biasd": biasd,
            }
        )
    return in_maps


def assemble(results):
    out = np.empty((B, O, H, W), np.float32)
    for core in range(8):
        b, ph = core // 2, core % 2
        arr = np.asarray(results[core]["outd"], np.float32).reshape(O, ROWS, W)
        out[b, :, ph * ROWS : (ph + 1) * ROWS, :] = arr
    return out


def kernel(x, offset, mask, weight, bias):
    from concourse.bass_utils import run_bass_kernel_spmd

    nc = get_nc()
    in_maps = make_in_maps(x, offset, mask, weight, bias)
    res = run_bass_kernel_spmd(nc, in_maps, core_ids=list(range(8)))
    return assemble(res.results)

